# revision 8
# baseline (speedup 1.0000x reference)
"""BatchTopK SAE forward pass on 8 Trainium2 NeuronCores.

Strategy (data-parallel over the batch, weights replicated):
  Launch 1 (per core): fp32 encoder  acts[j,b] = relu((x-b_dec) @ W_enc + b_enc)
      on a 512-row batch shard, laid out transposed [d_sae, batch_local] so the
      contraction dim is on partitions for both matmul phases.
  Host (gather point between launches): exact global top-(K*B) threshold from
      the device-produced acts (np.partition), index-ordered tie resolution,
      and exact per-row 32nd-largest dead-masked threshold for the aux loss.
  Launch 2 (per core): masked decode  x_rec = (acts>=t)*acts @ W_dec + b_dec
      and aux decode x_aux = (dead & acts>=tau_row)*acts @ W_dec, fp32r matmuls.
  Host: unshard/transpose outputs, apply rare tie patches, reduce the losses.
"""

import sys

sys.path.insert(0, "/opt/trn_rl_repo")

import numpy as np

B = 4096
D = 2048
J = 16384
K = 64
T = K * B
K_AUX = 32
N_DEAD = 10
AUX_PEN = 1.0 / 32.0
NCORES = 8
BL = B // NCORES  # 512 rows per core
DC = D // 128  # 16 contraction chunks (encoder)
JC = J // 128  # 128 d_sae chunks
MB = 4  # decoder m-chunks held in PSUM at once (x4 = 8 banks with aux)
NBLK = D // (128 * MB)  # 4 decoder blocks

_cache = {}


def _f32():
    import concourse.mybir as mybir

    return mybir.dt.float32


def _build_l1():
    import concourse.mybir as mybir
    from concourse.bacc import Bacc
    from concourse.tile import TileContext

    f32 = mybir.dt.float32
    nc = Bacc()
    xT = nc.dram_tensor("xT", [D, BL], f32, kind="ExternalInput")
    We = nc.dram_tensor("We", [D, J], f32, kind="ExternalInput")
    be = nc.dram_tensor("be", [J], f32, kind="ExternalInput")
    bd = nc.dram_tensor("bd", [D], f32, kind="ExternalInput")
    acts = nc.dram_tensor("acts", [J, BL], f32, kind="ExternalOutput")

    with TileContext(nc) as tc:
        with (
            tc.tile_pool(name="xpool", bufs=1) as xpool,
            tc.tile_pool(name="wpool", bufs=3) as wpool,
            tc.tile_pool(name="opool", bufs=3) as opool,
            tc.tile_pool(name="ppool", bufs=2, space="PSUM") as ppool,
        ):
            xs = xpool.tile([128, DC, BL], f32)
            nc.sync.dma_start(out=xs, in_=xT[:, :].rearrange("(c p) b -> p c b", p=128))
            bds = xpool.tile([128, DC], f32)
            nc.sync.dma_start(out=bds, in_=bd[:].rearrange("(c p) -> p c", p=128))
            bes = xpool.tile([128, JC], f32)
            nc.sync.dma_start(out=bes, in_=be[:].rearrange("(c p) -> p c", p=128))
            for c in range(DC):
                nc.vector.tensor_scalar_sub(xs[:, c, :], xs[:, c, :], bds[:, c : c + 1])
            for jc in range(JC):
                ws = wpool.tile([128, DC, 128], f32, tag="ws")
                nc.sync.dma_start(
                    out=ws,
                    in_=We[:, jc * 128 : (jc + 1) * 128].rearrange(
                        "(c p) j -> p c j", p=128
                    ),
                )
                acc = ppool.tile([128, BL], f32, tag="acc")
                for c in range(DC):
                    nc.tensor.matmul(
                        acc,
                        ws[:, c, :],
                        xs[:, c, :],
                        start=(c == 0),
                        stop=(c == DC - 1),
                    )
                ot = opool.tile([128, BL], f32, tag="ot")
                nc.scalar.activation(
                    ot,
                    acc,
                    mybir.ActivationFunctionType.Relu,
                    bias=bes[:, jc : jc + 1],
                    scale=1.0,
                )
                nc.sync.dma_start(out=acts[jc * 128 : (jc + 1) * 128, :], in_=ot)
    nc.finalize()
    return nc


def _build_l2():
    import concourse.mybir as mybir
    from concourse.bacc import Bacc
    from concourse.tile import TileContext

    f32 = mybir.dt.float32
    f32r = mybir.dt.float32r
    bf16 = mybir.dt.bfloat16
    nc = Bacc()
    acts = nc.dram_tensor("acts", [J, BL], f32, kind="ExternalInput")
    Wd = nc.dram_tensor("Wd", [J, D], f32, kind="ExternalInput")
    Wd16 = nc.dram_tensor("Wd16", [J, D], bf16, kind="ExternalInput")
    bd = nc.dram_tensor("bd", [D], f32, kind="ExternalInput")
    thr = nc.dram_tensor("thr", [128, 1], f32, kind="ExternalInput")
    tau = nc.dram_tensor("tau", [128, BL], f32, kind="ExternalInput")
    deadc = nc.dram_tensor("deadc", [128, JC], f32, kind="ExternalInput")
    topk_o = nc.dram_tensor("topk_o", [J, BL], f32, kind="ExternalOutput")
    xrec = nc.dram_tensor("xrec", [D, BL], f32, kind="ExternalOutput")
    xaux = nc.dram_tensor("xaux", [D, BL], f32, kind="ExternalOutput")

    with TileContext(nc) as tc:
        with (
            tc.tile_pool(name="cpool", bufs=1) as cpool,
            tc.tile_pool(name="apool", bufs=3) as apool,
            tc.tile_pool(name="wpool", bufs=3) as wpool,
            tc.tile_pool(name="opool", bufs=3) as opool,
            tc.tile_pool(name="ppool", bufs=1, space="PSUM") as ppool,
        ):
            thr_s = cpool.tile([128, 1], f32)
            nc.sync.dma_start(out=thr_s, in_=thr[:, :])
            tau_s = cpool.tile([128, BL], f32)
            nc.sync.dma_start(out=tau_s, in_=tau[:, :])
            deadc_s = cpool.tile([128, JC], f32)
            nc.sync.dma_start(out=deadc_s, in_=deadc[:, :])
            bds = cpool.tile([128, DC], f32)
            nc.sync.dma_start(out=bds, in_=bd[:].rearrange("(c p) -> p c", p=128))

            for blk in range(NBLK):
                rec_ps = [
                    ppool.tile([128, BL], f32, name=f"rec{mi}", tag=f"rec{mi}")
                    for mi in range(MB)
                ]
                aux_ps = [
                    ppool.tile([128, BL], f32, name=f"aux{mi}", tag=f"aux{mi}")
                    for mi in range(MB)
                ]
                for jc in range(JC):
                    at = apool.tile([128, BL], f32, tag="at")
                    nc.sync.dma_start(
                        out=at, in_=acts[jc * 128 : (jc + 1) * 128, :]
                    )
                    # topk = (acts >= t) * acts
                    tk = apool.tile([128, BL], f32r, tag="tk")
                    nc.vector.scalar_tensor_tensor(
                        tk,
                        at,
                        thr_s[:, 0:1],
                        at,
                        mybir.AluOpType.is_ge,
                        mybir.AluOpType.mult,
                    )
                    if blk == 0:
                        nc.sync.dma_start(
                            out=topk_o[jc * 128 : (jc + 1) * 128, :],
                            in_=tk.bitcast(f32),
                        )
                    # qm = (acts * dead[j]) >= tau[b] ; aux = qm * acts
                    qm = apool.tile([128, BL], f32, tag="qm")
                    nc.vector.scalar_tensor_tensor(
                        qm,
                        at,
                        deadc_s[:, jc : jc + 1],
                        tau_s,
                        mybir.AluOpType.mult,
                        mybir.AluOpType.is_ge,
                    )
                    ax = apool.tile([128, BL], bf16, tag="ax")
                    nc.gpsimd.tensor_mul(ax, qm, at)
                    wt = wpool.tile([128, MB * 128], f32r, tag="wt")
                    nc.sync.dma_start(
                        out=wt,
                        in_=Wd[
                            jc * 128 : (jc + 1) * 128,
                            blk * MB * 128 : (blk + 1) * MB * 128,
                        ].bitcast(f32r),
                    )
                    wt16 = wpool.tile([128, MB * 128], bf16, tag="wt16")
                    nc.sync.dma_start(
                        out=wt16,
                        in_=Wd16[
                            jc * 128 : (jc + 1) * 128,
                            blk * MB * 128 : (blk + 1) * MB * 128,
                        ],
                    )
                    for mi in range(MB):
                        w_r = wt[:, mi * 128 : (mi + 1) * 128]
                        nc.tensor.matmul(
                            rec_ps[mi], w_r, tk, start=(jc == 0), stop=(jc == JC - 1)
                        )
                        nc.tensor.matmul(
                            aux_ps[mi],
                            wt16[:, mi * 128 : (mi + 1) * 128],
                            ax,
                            start=(jc == 0),
                            stop=(jc == JC - 1),
                        )
                for mi in range(MB):
                    mc = blk * MB + mi
                    rt = opool.tile([128, BL], f32, tag="rt")
                    nc.scalar.activation(
                        rt,
                        rec_ps[mi],
                        mybir.ActivationFunctionType.Identity,
                        bias=bds[:, mc : mc + 1],
                        scale=1.0,
                    )
                    nc.sync.dma_start(out=xrec[mc * 128 : (mc + 1) * 128, :], in_=rt)
                    au = opool.tile([128, BL], f32, tag="au")
                    nc.scalar.copy(au, aux_ps[mi])
                    nc.sync.dma_start(out=xaux[mc * 128 : (mc + 1) * 128, :], in_=au)
    nc.finalize()
    return nc


def _get_kernels():
    if "l1" not in _cache:
        _cache["l1"] = _build_l1()
        _cache["l2"] = _build_l2()
    return _cache["l1"], _cache["l2"]


def _run_spmd(nc, in_maps, trace=False):
    from concourse.bass_utils import run_bass_kernel_spmd

    if trace:
        try:
            return run_bass_kernel_spmd(
                nc, in_maps, core_ids=list(range(NCORES)), trace=True
            )
        except Exception as e:  # profiling infra unavailable -> run untraced
            print(f"trace run failed ({type(e).__name__}: {e}); rerunning untraced")
    return run_bass_kernel_spmd(nc, in_maps, core_ids=list(range(NCORES)))


def _numpy_fallback(x, W_dec, b_dec, nbna, acts):
    """Exact numpy evaluation downstream of acts (degenerate inputs only)."""
    flat = acts.ravel()
    idx = np.argsort(-flat, kind="stable")[:T]
    topk = np.zeros_like(flat)
    topk[idx] = flat[idx]
    acts_topk = topk.reshape(acts.shape)
    x_rec = (acts_topk @ W_dec + b_dec).astype(np.float32)
    l2_loss = np.float32(np.mean((x_rec - x).astype(np.float64) ** 2))
    l1_norm = np.float32(acts_topk.sum(dtype=np.float64) / B)
    l0_norm = np.float32((acts_topk > 0).sum() / B)
    dead = nbna >= N_DEAD
    residual = x - x_rec
    masked = np.where(dead[None, :], acts, -1.0)
    sel = np.argsort(-masked, axis=1, kind="stable")[:, :K_AUX]
    acts_aux = np.zeros_like(acts)
    rows = np.arange(B)[:, None]
    acts_aux[rows, sel] = np.maximum(masked[rows, sel], 0.0)
    x_aux = (acts_aux @ W_dec).astype(np.float32)
    aux_l2 = np.float32(AUX_PEN * np.mean((x_aux - residual).astype(np.float64) ** 2))
    n_dead = int(dead.sum())
    aux_loss = aux_l2 if n_dead > 0 else np.float32(0.0)
    loss = np.float32(l2_loss + aux_loss)
    return (
        x_rec,
        acts_topk.astype(np.float32),
        loss,
        l2_loss,
        np.float32(0.0),
        l0_norm,
        l1_norm,
        aux_loss,
        np.int32((nbna > N_DEAD).sum()),
    )


def kernel(x, W_enc, W_dec, b_enc, b_dec, num_batches_not_active, _timing=None):
    x = np.ascontiguousarray(x, dtype=np.float32)
    W_enc = np.ascontiguousarray(W_enc, dtype=np.float32)
    W_dec = np.ascontiguousarray(W_dec, dtype=np.float32)
    b_enc = np.ascontiguousarray(b_enc, dtype=np.float32)
    b_dec = np.ascontiguousarray(b_dec, dtype=np.float32)
    nbna = np.asarray(num_batches_not_active)

    l1, l2 = _get_kernels()

    # ---------------- launch 1: encoder ----------------
    xT = [np.ascontiguousarray(x[i * BL : (i + 1) * BL].T) for i in range(NCORES)]
    in1 = [{"xT": xT[i], "We": W_enc, "be": b_enc, "bd": b_dec} for i in range(NCORES)]
    r1 = _run_spmd(l1, in1, trace=bool(_timing is not None))
    acts_sh = [r1.results[i]["acts"] for i in range(NCORES)]  # [J, BL] each
    if _timing is not None:
        _timing.append(("l1", r1.exec_time_ns))

    # ---------------- host: exact global top-T threshold ----------------
    allv = np.concatenate([a.ravel() for a in acts_sh])
    t = np.partition(allv, allv.size - T)[allv.size - T]
    count_gt = int((allv > t).sum())
    needed = T - count_gt  # >= 1 by construction
    del allv
    if not (t > 0.0):
        # Degenerate input (threshold at/below relu floor): fall back to a
        # plain numpy evaluation of the whole forward pass for correctness.
        return _numpy_fallback(
            x, W_dec, b_dec, nbna, np.concatenate([a.T for a in acts_sh], axis=0)
        )
    # tie positions in reference flat order: flat = (global_row)*J + j
    tie_flat = []
    for i in range(NCORES):
        jj, bb = np.nonzero(acts_sh[i] == t)
        rows = i * BL + bb
        tie_flat.extend(zip(rows * J + jj, rows, jj))
    tie_flat.sort()
    drops = tie_flat[needed:]  # (flat, row, j) to zero out, usually empty
    count_kept = count_gt + needed

    # ---------------- host: per-row aux threshold (exact 32nd largest) ----
    dead = (nbna >= N_DEAD).astype(np.float32)  # [J]
    tau = np.empty((B,), dtype=np.float32)
    for i in range(NCORES):
        masked = np.where(dead[:, None] > 0, acts_sh[i], -1.0)  # [J, BL]
        tau[i * BL : (i + 1) * BL] = np.partition(masked, J - K_AUX, axis=0)[
            J - K_AUX, :
        ]
        del masked
    tau = np.maximum(tau, np.float32(1e-30))  # tau<=0 => select all positive dead

    # ---------------- launch 2: masked decode + aux decode ----------------
    import ml_dtypes

    W_dec16 = W_dec.astype(ml_dtypes.bfloat16)
    thr_in = np.full((128, 1), t, dtype=np.float32)
    deadc_in = np.ascontiguousarray(dead.reshape(JC, 128).T)  # [128, JC]
    in2 = []
    for i in range(NCORES):
        tau_in = np.ascontiguousarray(
            np.broadcast_to(tau[i * BL : (i + 1) * BL], (128, BL))
        )
        in2.append(
            {
                "acts": acts_sh[i],
                "Wd": W_dec,
                "Wd16": W_dec16,
                "bd": b_dec,
                "thr": thr_in,
                "tau": tau_in,
                "deadc": deadc_in,
            }
        )
    r2 = _run_spmd(l2, in2, trace=bool(_timing is not None))
    if _timing is not None:
        _timing.append(("l2", r2.exec_time_ns))

    # ---------------- host: unshard + tie patches + losses ----------------
    acts_topk = np.empty((B, J), dtype=np.float32)
    x_rec = np.empty((B, D), dtype=np.float32)
    x_aux = np.empty((B, D), dtype=np.float32)
    for i in range(NCORES):
        acts_topk[i * BL : (i + 1) * BL] = r2.results[i]["topk_o"].T
        x_rec[i * BL : (i + 1) * BL] = r2.results[i]["xrec"].T
        x_aux[i * BL : (i + 1) * BL] = r2.results[i]["xaux"].T
    for _, row, j in drops:
        acts_topk[row, j] = 0.0
        x_rec[row] -= t * W_dec[j]

    l2_loss = np.float32(np.mean((x_rec - x).astype(np.float64) ** 2))
    l1_norm = np.float32(acts_topk.sum(dtype=np.float64) / B)
    l1_loss = np.float32(0.0)
    l0_norm = np.float32(count_kept / B)
    residual = x - x_rec
    aux_l2 = np.float32(
        AUX_PEN * np.mean((x_aux - residual).astype(np.float64) ** 2)
    )
    n_dead = int((nbna >= N_DEAD).sum())
    aux_loss = aux_l2 if n_dead > 0 else np.float32(0.0)
    loss = np.float32(l2_loss + l1_loss + aux_loss)
    num_dead_features = np.int32((nbna > N_DEAD).sum())

    return (
        x_rec,
        acts_topk,
        loss,
        l2_loss,
        l1_loss,
        l0_norm,
        l1_norm,
        aux_loss,
        num_dead_features,
    )


# revision 9
# speedup vs baseline: 1.0678x; 1.0678x over previous
"""BatchTopK SAE forward pass on 8 Trainium2 NeuronCores.

Strategy (data-parallel over the batch, weights replicated):
  Launch 1 (per core): fp32 encoder  acts[j,b] = relu((x-b_dec) @ W_enc + b_enc)
      on a 512-row batch shard, laid out transposed [d_sae, batch_local] so the
      contraction dim is on partitions for both matmul phases.
  Host (gather point between launches): exact global top-(K*B) threshold from
      the device-produced acts (np.partition), index-ordered tie resolution,
      and exact per-row 32nd-largest dead-masked threshold for the aux loss.
  Launch 2 (per core): masked decode  x_rec = (acts>=t)*acts @ W_dec + b_dec
      and aux decode x_aux = (dead & acts>=tau_row)*acts @ W_dec, fp32r matmuls.
  Host: unshard/transpose outputs, apply rare tie patches, reduce the losses.
"""

import sys

sys.path.insert(0, "/opt/trn_rl_repo")

import numpy as np

B = 4096
D = 2048
J = 16384
K = 64
T = K * B
K_AUX = 32
N_DEAD = 10
AUX_PEN = 1.0 / 32.0
NCORES = 8
BL = B // NCORES  # 512 rows per core
DC = D // 128  # 16 contraction chunks (encoder)
JC = J // 128  # 128 d_sae chunks
MB = 4  # decoder m-chunks held in PSUM at once (x4 = 8 banks with aux)
NBLK = D // (128 * MB)  # 4 decoder blocks

_cache = {}


def _f32():
    import concourse.mybir as mybir

    return mybir.dt.float32


def _build_l1():
    import concourse.mybir as mybir
    from concourse.bacc import Bacc
    from concourse.tile import TileContext

    f32 = mybir.dt.float32
    nc = Bacc()
    xT = nc.dram_tensor("xT", [D, BL], f32, kind="ExternalInput")
    We = nc.dram_tensor("We", [D, J], f32, kind="ExternalInput")
    be = nc.dram_tensor("be", [J], f32, kind="ExternalInput")
    bd = nc.dram_tensor("bd", [D], f32, kind="ExternalInput")
    acts = nc.dram_tensor("acts", [J, BL], f32, kind="ExternalOutput")

    with TileContext(nc) as tc:
        with (
            tc.tile_pool(name="xpool", bufs=1) as xpool,
            tc.tile_pool(name="wpool", bufs=3) as wpool,
            tc.tile_pool(name="opool", bufs=3) as opool,
            tc.tile_pool(name="ppool", bufs=2, space="PSUM") as ppool,
        ):
            xs = xpool.tile([128, DC, BL], f32)
            nc.sync.dma_start(out=xs, in_=xT[:, :].rearrange("(c p) b -> p c b", p=128))
            bds = xpool.tile([128, DC], f32)
            nc.sync.dma_start(out=bds, in_=bd[:].rearrange("(c p) -> p c", p=128))
            bes = xpool.tile([128, JC], f32)
            nc.sync.dma_start(out=bes, in_=be[:].rearrange("(c p) -> p c", p=128))
            for c in range(DC):
                nc.vector.tensor_scalar_sub(xs[:, c, :], xs[:, c, :], bds[:, c : c + 1])
            for jc in range(JC):
                ws = wpool.tile([128, DC, 128], f32, tag="ws")
                nc.sync.dma_start(
                    out=ws,
                    in_=We[:, jc * 128 : (jc + 1) * 128].rearrange(
                        "(c p) j -> p c j", p=128
                    ),
                )
                acc = ppool.tile([128, BL], f32, tag="acc")
                for c in range(DC):
                    nc.tensor.matmul(
                        acc,
                        ws[:, c, :],
                        xs[:, c, :],
                        start=(c == 0),
                        stop=(c == DC - 1),
                    )
                ot = opool.tile([128, BL], f32, tag="ot")
                nc.scalar.activation(
                    ot,
                    acc,
                    mybir.ActivationFunctionType.Relu,
                    bias=bes[:, jc : jc + 1],
                    scale=1.0,
                )
                nc.sync.dma_start(out=acts[jc * 128 : (jc + 1) * 128, :], in_=ot)
    nc.finalize()
    return nc


def _build_l2():
    import concourse.mybir as mybir
    from concourse.bacc import Bacc
    from concourse.tile import TileContext

    f32 = mybir.dt.float32
    f32r = mybir.dt.float32r
    bf16 = mybir.dt.bfloat16
    nc = Bacc()
    acts = nc.dram_tensor("acts", [J, BL], f32, kind="ExternalInput")
    Wd = nc.dram_tensor("Wd", [J, D], f32, kind="ExternalInput")
    Wd16 = nc.dram_tensor("Wd16", [J, D], bf16, kind="ExternalInput")
    bd = nc.dram_tensor("bd", [D], f32, kind="ExternalInput")
    thr = nc.dram_tensor("thr", [128, 1], f32, kind="ExternalInput")
    tau = nc.dram_tensor("tau", [128, BL], f32, kind="ExternalInput")
    deadc = nc.dram_tensor("deadc", [128, JC], f32, kind="ExternalInput")
    topk_o = nc.dram_tensor("topk_o", [J, BL], f32, kind="ExternalOutput")
    xrec = nc.dram_tensor("xrec", [D, BL], f32, kind="ExternalOutput")
    xaux = nc.dram_tensor("xaux", [D, BL], f32, kind="ExternalOutput")

    with TileContext(nc) as tc:
        with (
            tc.tile_pool(name="cpool", bufs=1) as cpool,
            tc.tile_pool(name="apool", bufs=3) as apool,
            tc.tile_pool(name="wpool", bufs=3) as wpool,
            tc.tile_pool(name="opool", bufs=3) as opool,
            tc.tile_pool(name="ppool", bufs=1, space="PSUM") as ppool,
        ):
            thr_s = cpool.tile([128, 1], f32)
            nc.sync.dma_start(out=thr_s, in_=thr[:, :])
            tau_s = cpool.tile([128, BL], f32)
            nc.sync.dma_start(out=tau_s, in_=tau[:, :])
            deadc_s = cpool.tile([128, JC], f32)
            nc.sync.dma_start(out=deadc_s, in_=deadc[:, :])
            bds = cpool.tile([128, DC], f32)
            nc.sync.dma_start(out=bds, in_=bd[:].rearrange("(c p) -> p c", p=128))

            for blk in range(NBLK):
                rec_ps = [
                    ppool.tile([128, BL], f32, name=f"rec{mi}", tag=f"rec{mi}")
                    for mi in range(MB)
                ]
                aux_ps = [
                    ppool.tile([128, BL], f32, name=f"aux{mi}", tag=f"aux{mi}")
                    for mi in range(MB)
                ]
                for jc in range(JC):
                    at = apool.tile([128, BL], f32, tag="at")
                    nc.sync.dma_start(
                        out=at, in_=acts[jc * 128 : (jc + 1) * 128, :]
                    )
                    # topk = (acts >= t) * acts
                    tk = apool.tile([128, BL], f32r, tag="tk")
                    nc.vector.scalar_tensor_tensor(
                        tk,
                        at,
                        thr_s[:, 0:1],
                        at,
                        mybir.AluOpType.is_ge,
                        mybir.AluOpType.mult,
                    )
                    if blk == 0:
                        nc.sync.dma_start(
                            out=topk_o[jc * 128 : (jc + 1) * 128, :],
                            in_=tk.bitcast(f32),
                        )
                    # qm = (acts * dead[j]) >= tau[b] ; aux = qm * acts
                    qm = apool.tile([128, BL], f32, tag="qm")
                    nc.vector.scalar_tensor_tensor(
                        qm,
                        at,
                        deadc_s[:, jc : jc + 1],
                        tau_s,
                        mybir.AluOpType.mult,
                        mybir.AluOpType.is_ge,
                    )
                    ax = apool.tile([128, BL], bf16, tag="ax")
                    nc.vector.tensor_mul(ax, qm, at)
                    wt = wpool.tile([128, MB * 128], f32r, tag="wt")
                    nc.sync.dma_start(
                        out=wt,
                        in_=Wd[
                            jc * 128 : (jc + 1) * 128,
                            blk * MB * 128 : (blk + 1) * MB * 128,
                        ].bitcast(f32r),
                    )
                    wt16 = wpool.tile([128, MB * 128], bf16, tag="wt16")
                    nc.sync.dma_start(
                        out=wt16,
                        in_=Wd16[
                            jc * 128 : (jc + 1) * 128,
                            blk * MB * 128 : (blk + 1) * MB * 128,
                        ],
                    )
                    for mi in range(MB):
                        w_r = wt[:, mi * 128 : (mi + 1) * 128]
                        nc.tensor.matmul(
                            rec_ps[mi], w_r, tk, start=(jc == 0), stop=(jc == JC - 1)
                        )
                        nc.tensor.matmul(
                            aux_ps[mi],
                            wt16[:, mi * 128 : (mi + 1) * 128],
                            ax,
                            start=(jc == 0),
                            stop=(jc == JC - 1),
                        )
                for mi in range(MB):
                    mc = blk * MB + mi
                    rt = opool.tile([128, BL], f32, tag="rt")
                    nc.scalar.activation(
                        rt,
                        rec_ps[mi],
                        mybir.ActivationFunctionType.Identity,
                        bias=bds[:, mc : mc + 1],
                        scale=1.0,
                    )
                    nc.sync.dma_start(out=xrec[mc * 128 : (mc + 1) * 128, :], in_=rt)
                    au = opool.tile([128, BL], f32, tag="au")
                    nc.scalar.copy(au, aux_ps[mi])
                    nc.sync.dma_start(out=xaux[mc * 128 : (mc + 1) * 128, :], in_=au)
    nc.finalize()
    return nc


def _get_kernels():
    if "l1" not in _cache:
        _cache["l1"] = _build_l1()
        _cache["l2"] = _build_l2()
    return _cache["l1"], _cache["l2"]


def _run_spmd(nc, in_maps, trace=False):
    from concourse.bass_utils import run_bass_kernel_spmd

    if trace:
        try:
            return run_bass_kernel_spmd(
                nc, in_maps, core_ids=list(range(NCORES)), trace=True
            )
        except Exception as e:  # profiling infra unavailable -> run untraced
            print(f"trace run failed ({type(e).__name__}: {e}); rerunning untraced")
    return run_bass_kernel_spmd(nc, in_maps, core_ids=list(range(NCORES)))


def _numpy_fallback(x, W_dec, b_dec, nbna, acts):
    """Exact numpy evaluation downstream of acts (degenerate inputs only)."""
    flat = acts.ravel()
    idx = np.argsort(-flat, kind="stable")[:T]
    topk = np.zeros_like(flat)
    topk[idx] = flat[idx]
    acts_topk = topk.reshape(acts.shape)
    x_rec = (acts_topk @ W_dec + b_dec).astype(np.float32)
    l2_loss = np.float32(np.mean((x_rec - x).astype(np.float64) ** 2))
    l1_norm = np.float32(acts_topk.sum(dtype=np.float64) / B)
    l0_norm = np.float32((acts_topk > 0).sum() / B)
    dead = nbna >= N_DEAD
    residual = x - x_rec
    masked = np.where(dead[None, :], acts, -1.0)
    sel = np.argsort(-masked, axis=1, kind="stable")[:, :K_AUX]
    acts_aux = np.zeros_like(acts)
    rows = np.arange(B)[:, None]
    acts_aux[rows, sel] = np.maximum(masked[rows, sel], 0.0)
    x_aux = (acts_aux @ W_dec).astype(np.float32)
    aux_l2 = np.float32(AUX_PEN * np.mean((x_aux - residual).astype(np.float64) ** 2))
    n_dead = int(dead.sum())
    aux_loss = aux_l2 if n_dead > 0 else np.float32(0.0)
    loss = np.float32(l2_loss + aux_loss)
    return (
        x_rec,
        acts_topk.astype(np.float32),
        loss,
        l2_loss,
        np.float32(0.0),
        l0_norm,
        l1_norm,
        aux_loss,
        np.int32((nbna > N_DEAD).sum()),
    )


def kernel(x, W_enc, W_dec, b_enc, b_dec, num_batches_not_active, _timing=None):
    x = np.ascontiguousarray(x, dtype=np.float32)
    W_enc = np.ascontiguousarray(W_enc, dtype=np.float32)
    W_dec = np.ascontiguousarray(W_dec, dtype=np.float32)
    b_enc = np.ascontiguousarray(b_enc, dtype=np.float32)
    b_dec = np.ascontiguousarray(b_dec, dtype=np.float32)
    nbna = np.asarray(num_batches_not_active)

    l1, l2 = _get_kernels()

    # ---------------- launch 1: encoder ----------------
    xT = [np.ascontiguousarray(x[i * BL : (i + 1) * BL].T) for i in range(NCORES)]
    in1 = [{"xT": xT[i], "We": W_enc, "be": b_enc, "bd": b_dec} for i in range(NCORES)]
    r1 = _run_spmd(l1, in1, trace=bool(_timing is not None))
    acts_sh = [r1.results[i]["acts"] for i in range(NCORES)]  # [J, BL] each
    if _timing is not None:
        _timing.append(("l1", r1.exec_time_ns))

    # ---------------- host: exact global top-T threshold ----------------
    allv = np.concatenate([a.ravel() for a in acts_sh])
    t = np.partition(allv, allv.size - T)[allv.size - T]
    count_gt = int((allv > t).sum())
    needed = T - count_gt  # >= 1 by construction
    del allv
    if not (t > 0.0):
        # Degenerate input (threshold at/below relu floor): fall back to a
        # plain numpy evaluation of the whole forward pass for correctness.
        return _numpy_fallback(
            x, W_dec, b_dec, nbna, np.concatenate([a.T for a in acts_sh], axis=0)
        )
    # tie positions in reference flat order: flat = (global_row)*J + j
    tie_flat = []
    for i in range(NCORES):
        jj, bb = np.nonzero(acts_sh[i] == t)
        rows = i * BL + bb
        tie_flat.extend(zip(rows * J + jj, rows, jj))
    tie_flat.sort()
    drops = tie_flat[needed:]  # (flat, row, j) to zero out, usually empty
    count_kept = count_gt + needed

    # ---------------- host: per-row aux threshold (exact 32nd largest) ----
    dead = (nbna >= N_DEAD).astype(np.float32)  # [J]
    tau = np.empty((B,), dtype=np.float32)
    for i in range(NCORES):
        masked = np.where(dead[:, None] > 0, acts_sh[i], -1.0)  # [J, BL]
        tau[i * BL : (i + 1) * BL] = np.partition(masked, J - K_AUX, axis=0)[
            J - K_AUX, :
        ]
        del masked
    tau = np.maximum(tau, np.float32(1e-30))  # tau<=0 => select all positive dead

    # ---------------- launch 2: masked decode + aux decode ----------------
    import ml_dtypes

    W_dec16 = W_dec.astype(ml_dtypes.bfloat16)
    thr_in = np.full((128, 1), t, dtype=np.float32)
    deadc_in = np.ascontiguousarray(dead.reshape(JC, 128).T)  # [128, JC]
    in2 = []
    for i in range(NCORES):
        tau_in = np.ascontiguousarray(
            np.broadcast_to(tau[i * BL : (i + 1) * BL], (128, BL))
        )
        in2.append(
            {
                "acts": acts_sh[i],
                "Wd": W_dec,
                "Wd16": W_dec16,
                "bd": b_dec,
                "thr": thr_in,
                "tau": tau_in,
                "deadc": deadc_in,
            }
        )
    r2 = _run_spmd(l2, in2, trace=bool(_timing is not None))
    if _timing is not None:
        _timing.append(("l2", r2.exec_time_ns))

    # ---------------- host: unshard + tie patches + losses ----------------
    acts_topk = np.empty((B, J), dtype=np.float32)
    x_rec = np.empty((B, D), dtype=np.float32)
    x_aux = np.empty((B, D), dtype=np.float32)
    for i in range(NCORES):
        acts_topk[i * BL : (i + 1) * BL] = r2.results[i]["topk_o"].T
        x_rec[i * BL : (i + 1) * BL] = r2.results[i]["xrec"].T
        x_aux[i * BL : (i + 1) * BL] = r2.results[i]["xaux"].T
    for _, row, j in drops:
        acts_topk[row, j] = 0.0
        x_rec[row] -= t * W_dec[j]

    l2_loss = np.float32(np.mean((x_rec - x).astype(np.float64) ** 2))
    l1_norm = np.float32(acts_topk.sum(dtype=np.float64) / B)
    l1_loss = np.float32(0.0)
    l0_norm = np.float32(count_kept / B)
    residual = x - x_rec
    aux_l2 = np.float32(
        AUX_PEN * np.mean((x_aux - residual).astype(np.float64) ** 2)
    )
    n_dead = int((nbna >= N_DEAD).sum())
    aux_loss = aux_l2 if n_dead > 0 else np.float32(0.0)
    loss = np.float32(l2_loss + l1_loss + aux_loss)
    num_dead_features = np.int32((nbna > N_DEAD).sum())

    return (
        x_rec,
        acts_topk,
        loss,
        l2_loss,
        l1_loss,
        l0_norm,
        l1_norm,
        aux_loss,
        num_dead_features,
    )


# revision 10
# speedup vs baseline: 1.0772x; 1.0088x over previous
"""BatchTopK SAE forward pass on 8 Trainium2 NeuronCores.

Strategy (data-parallel over the batch, weights replicated):
  Launch 1 (per core): fp32 encoder  acts[j,b] = relu((x-b_dec) @ W_enc + b_enc)
      on a 512-row batch shard, laid out transposed [d_sae, batch_local] so the
      contraction dim is on partitions for both matmul phases.
  Host (gather point between launches): exact global top-(K*B) threshold from
      the device-produced acts (np.partition), index-ordered tie resolution,
      and exact per-row 32nd-largest dead-masked threshold for the aux loss.
  Launch 2 (per core): masked decode  x_rec = (acts>=t)*acts @ W_dec + b_dec
      and aux decode x_aux = (dead & acts>=tau_row)*acts @ W_dec, fp32r matmuls.
  Host: unshard/transpose outputs, apply rare tie patches, reduce the losses.
"""

import sys

sys.path.insert(0, "/opt/trn_rl_repo")

import numpy as np

B = 4096
D = 2048
J = 16384
K = 64
T = K * B
K_AUX = 32
N_DEAD = 10
AUX_PEN = 1.0 / 32.0
NCORES = 8
BL = B // NCORES  # 512 rows per core
DC = D // 128  # 16 contraction chunks (encoder)
JC = J // 128  # 128 d_sae chunks
MB = 4  # decoder m-chunks held in PSUM at once (x4 = 8 banks with aux)
NBLK = D // (128 * MB)  # 4 decoder blocks

_cache = {}


def _f32():
    import concourse.mybir as mybir

    return mybir.dt.float32


def _build_l1():
    import concourse.mybir as mybir
    from concourse.bacc import Bacc
    from concourse.tile import TileContext

    f32 = mybir.dt.float32
    nc = Bacc()
    xT = nc.dram_tensor("xT", [D, BL], f32, kind="ExternalInput")
    We = nc.dram_tensor("We", [D, J], f32, kind="ExternalInput")
    be = nc.dram_tensor("be", [J], f32, kind="ExternalInput")
    bd = nc.dram_tensor("bd", [D], f32, kind="ExternalInput")
    acts = nc.dram_tensor("acts", [J, BL], f32, kind="ExternalOutput")

    with TileContext(nc) as tc:
        with (
            tc.tile_pool(name="xpool", bufs=1) as xpool,
            tc.tile_pool(name="wpool", bufs=3) as wpool,
            tc.tile_pool(name="opool", bufs=3) as opool,
            tc.tile_pool(name="ppool", bufs=2, space="PSUM") as ppool,
        ):
            xs = xpool.tile([128, DC, BL], f32)
            nc.sync.dma_start(out=xs, in_=xT[:, :].rearrange("(c p) b -> p c b", p=128))
            bds = xpool.tile([128, DC], f32)
            nc.sync.dma_start(out=bds, in_=bd[:].rearrange("(c p) -> p c", p=128))
            bes = xpool.tile([128, JC], f32)
            nc.sync.dma_start(out=bes, in_=be[:].rearrange("(c p) -> p c", p=128))
            for c in range(DC):
                nc.vector.tensor_scalar_sub(xs[:, c, :], xs[:, c, :], bds[:, c : c + 1])
            for jc in range(JC):
                ws = wpool.tile([128, DC, 128], f32, tag="ws")
                nc.sync.dma_start(
                    out=ws,
                    in_=We[:, jc * 128 : (jc + 1) * 128].rearrange(
                        "(c p) j -> p c j", p=128
                    ),
                )
                acc = ppool.tile([128, BL], f32, tag="acc")
                for c in range(DC):
                    nc.tensor.matmul(
                        acc,
                        ws[:, c, :],
                        xs[:, c, :],
                        start=(c == 0),
                        stop=(c == DC - 1),
                    )
                ot = opool.tile([128, BL], f32, tag="ot")
                nc.scalar.activation(
                    ot,
                    acc,
                    mybir.ActivationFunctionType.Relu,
                    bias=bes[:, jc : jc + 1],
                    scale=1.0,
                )
                nc.sync.dma_start(out=acts[jc * 128 : (jc + 1) * 128, :], in_=ot)
    nc.finalize()
    return nc


def _build_l2():
    import concourse.mybir as mybir
    from concourse.bacc import Bacc
    from concourse.tile import TileContext

    f32 = mybir.dt.float32
    f32r = mybir.dt.float32r
    nc = Bacc()
    acts = nc.dram_tensor("acts", [J, BL], f32, kind="ExternalInput")
    Wd = nc.dram_tensor("Wd", [J, D], f32, kind="ExternalInput")
    bd = nc.dram_tensor("bd", [D], f32, kind="ExternalInput")
    thr = nc.dram_tensor("thr", [128, 1], f32, kind="ExternalInput")
    tau = nc.dram_tensor("tau", [128, BL], f32, kind="ExternalInput")
    deadc = nc.dram_tensor("deadc", [128, JC], f32, kind="ExternalInput")
    topk_o = nc.dram_tensor("topk_o", [J, BL], f32, kind="ExternalOutput")
    xrec = nc.dram_tensor("xrec", [D, BL], f32, kind="ExternalOutput")
    xaux = nc.dram_tensor("xaux", [D, BL], f32, kind="ExternalOutput")

    with TileContext(nc) as tc:
        with (
            tc.tile_pool(name="cpool", bufs=1) as cpool,
            tc.tile_pool(name="apool", bufs=3) as apool,
            tc.tile_pool(name="wpool", bufs=3) as wpool,
            tc.tile_pool(name="opool", bufs=3) as opool,
            tc.tile_pool(name="ppool", bufs=1, space="PSUM") as ppool,
        ):
            thr_s = cpool.tile([128, 1], f32)
            nc.sync.dma_start(out=thr_s, in_=thr[:, :])
            tau_s = cpool.tile([128, BL], f32)
            nc.sync.dma_start(out=tau_s, in_=tau[:, :])
            deadc_s = cpool.tile([128, JC], f32)
            nc.sync.dma_start(out=deadc_s, in_=deadc[:, :])
            bds = cpool.tile([128, DC], f32)
            nc.sync.dma_start(out=bds, in_=bd[:].rearrange("(c p) -> p c", p=128))

            for blk in range(NBLK):
                rec_ps = [
                    ppool.tile([128, BL], f32, name=f"rec{mi}", tag=f"rec{mi}")
                    for mi in range(MB)
                ]
                aux_ps = [
                    ppool.tile([128, BL], f32, name=f"aux{mi}", tag=f"aux{mi}")
                    for mi in range(MB)
                ]
                for jc in range(JC):
                    at = apool.tile([128, BL], f32, tag="at")
                    nc.sync.dma_start(
                        out=at, in_=acts[jc * 128 : (jc + 1) * 128, :]
                    )
                    # topk = (acts >= t) * acts
                    tk = apool.tile([128, BL], f32r, tag="tk")
                    nc.vector.scalar_tensor_tensor(
                        tk,
                        at,
                        thr_s[:, 0:1],
                        at,
                        mybir.AluOpType.is_ge,
                        mybir.AluOpType.mult,
                    )
                    if blk == 0:
                        nc.sync.dma_start(
                            out=topk_o[jc * 128 : (jc + 1) * 128, :],
                            in_=tk.bitcast(f32),
                        )
                    # qm = (acts * dead[j]) >= tau[b] ; aux = qm * acts
                    qm = apool.tile([128, BL], f32, tag="qm")
                    nc.vector.scalar_tensor_tensor(
                        qm,
                        at,
                        deadc_s[:, jc : jc + 1],
                        tau_s,
                        mybir.AluOpType.mult,
                        mybir.AluOpType.is_ge,
                    )
                    ax = apool.tile([128, BL], f32r, tag="ax")
                    nc.vector.tensor_mul(ax, qm, at)
                    wt = wpool.tile([128, MB * 128], f32r, tag="wt")
                    nc.sync.dma_start(
                        out=wt,
                        in_=Wd[
                            jc * 128 : (jc + 1) * 128,
                            blk * MB * 128 : (blk + 1) * MB * 128,
                        ].bitcast(f32r),
                    )
                    for mi in range(MB):
                        w_r = wt[:, mi * 128 : (mi + 1) * 128]
                        nc.tensor.matmul(
                            rec_ps[mi], w_r, tk, start=(jc == 0), stop=(jc == JC - 1)
                        )
                        nc.tensor.matmul(
                            aux_ps[mi], w_r, ax, start=(jc == 0), stop=(jc == JC - 1)
                        )
                for mi in range(MB):
                    mc = blk * MB + mi
                    rt = opool.tile([128, BL], f32, tag="rt")
                    nc.scalar.activation(
                        rt,
                        rec_ps[mi],
                        mybir.ActivationFunctionType.Identity,
                        bias=bds[:, mc : mc + 1],
                        scale=1.0,
                    )
                    nc.sync.dma_start(out=xrec[mc * 128 : (mc + 1) * 128, :], in_=rt)
                    au = opool.tile([128, BL], f32, tag="au")
                    nc.scalar.copy(au, aux_ps[mi])
                    nc.sync.dma_start(out=xaux[mc * 128 : (mc + 1) * 128, :], in_=au)
    nc.finalize()
    return nc


def _get_kernels():
    if "l1" not in _cache:
        _cache["l1"] = _build_l1()
        _cache["l2"] = _build_l2()
    return _cache["l1"], _cache["l2"]


def _run_spmd(nc, in_maps, trace=False):
    from concourse.bass_utils import run_bass_kernel_spmd

    if trace:
        try:
            return run_bass_kernel_spmd(
                nc, in_maps, core_ids=list(range(NCORES)), trace=True
            )
        except Exception as e:  # profiling infra unavailable -> run untraced
            print(f"trace run failed ({type(e).__name__}: {e}); rerunning untraced")
    return run_bass_kernel_spmd(nc, in_maps, core_ids=list(range(NCORES)))


def _numpy_fallback(x, W_dec, b_dec, nbna, acts):
    """Exact numpy evaluation downstream of acts (degenerate inputs only)."""
    flat = acts.ravel()
    idx = np.argsort(-flat, kind="stable")[:T]
    topk = np.zeros_like(flat)
    topk[idx] = flat[idx]
    acts_topk = topk.reshape(acts.shape)
    x_rec = (acts_topk @ W_dec + b_dec).astype(np.float32)
    l2_loss = np.float32(np.mean((x_rec - x).astype(np.float64) ** 2))
    l1_norm = np.float32(acts_topk.sum(dtype=np.float64) / B)
    l0_norm = np.float32((acts_topk > 0).sum() / B)
    dead = nbna >= N_DEAD
    residual = x - x_rec
    masked = np.where(dead[None, :], acts, -1.0)
    sel = np.argsort(-masked, axis=1, kind="stable")[:, :K_AUX]
    acts_aux = np.zeros_like(acts)
    rows = np.arange(B)[:, None]
    acts_aux[rows, sel] = np.maximum(masked[rows, sel], 0.0)
    x_aux = (acts_aux @ W_dec).astype(np.float32)
    aux_l2 = np.float32(AUX_PEN * np.mean((x_aux - residual).astype(np.float64) ** 2))
    n_dead = int(dead.sum())
    aux_loss = aux_l2 if n_dead > 0 else np.float32(0.0)
    loss = np.float32(l2_loss + aux_loss)
    return (
        x_rec,
        acts_topk.astype(np.float32),
        loss,
        l2_loss,
        np.float32(0.0),
        l0_norm,
        l1_norm,
        aux_loss,
        np.int32((nbna > N_DEAD).sum()),
    )


def kernel(x, W_enc, W_dec, b_enc, b_dec, num_batches_not_active, _timing=None):
    x = np.ascontiguousarray(x, dtype=np.float32)
    W_enc = np.ascontiguousarray(W_enc, dtype=np.float32)
    W_dec = np.ascontiguousarray(W_dec, dtype=np.float32)
    b_enc = np.ascontiguousarray(b_enc, dtype=np.float32)
    b_dec = np.ascontiguousarray(b_dec, dtype=np.float32)
    nbna = np.asarray(num_batches_not_active)

    l1, l2 = _get_kernels()

    # ---------------- launch 1: encoder ----------------
    xT = [np.ascontiguousarray(x[i * BL : (i + 1) * BL].T) for i in range(NCORES)]
    in1 = [{"xT": xT[i], "We": W_enc, "be": b_enc, "bd": b_dec} for i in range(NCORES)]
    r1 = _run_spmd(l1, in1, trace=bool(_timing is not None))
    acts_sh = [r1.results[i]["acts"] for i in range(NCORES)]  # [J, BL] each
    if _timing is not None:
        _timing.append(("l1", r1.exec_time_ns))

    # ---------------- host: exact global top-T threshold ----------------
    allv = np.concatenate([a.ravel() for a in acts_sh])
    t = np.partition(allv, allv.size - T)[allv.size - T]
    count_gt = int((allv > t).sum())
    needed = T - count_gt  # >= 1 by construction
    del allv
    if not (t > 0.0):
        # Degenerate input (threshold at/below relu floor): fall back to a
        # plain numpy evaluation of the whole forward pass for correctness.
        return _numpy_fallback(
            x, W_dec, b_dec, nbna, np.concatenate([a.T for a in acts_sh], axis=0)
        )
    # tie positions in reference flat order: flat = (global_row)*J + j
    tie_flat = []
    for i in range(NCORES):
        jj, bb = np.nonzero(acts_sh[i] == t)
        rows = i * BL + bb
        tie_flat.extend(zip(rows * J + jj, rows, jj))
    tie_flat.sort()
    drops = tie_flat[needed:]  # (flat, row, j) to zero out, usually empty
    count_kept = count_gt + needed

    # ---------------- host: per-row aux threshold (exact 32nd largest) ----
    dead = (nbna >= N_DEAD).astype(np.float32)  # [J]
    tau = np.empty((B,), dtype=np.float32)
    for i in range(NCORES):
        masked = np.where(dead[:, None] > 0, acts_sh[i], -1.0)  # [J, BL]
        tau[i * BL : (i + 1) * BL] = np.partition(masked, J - K_AUX, axis=0)[
            J - K_AUX, :
        ]
        del masked
    tau = np.maximum(tau, np.float32(1e-30))  # tau<=0 => select all positive dead

    # ---------------- launch 2: masked decode + aux decode ----------------
    thr_in = np.full((128, 1), t, dtype=np.float32)
    deadc_in = np.ascontiguousarray(dead.reshape(JC, 128).T)  # [128, JC]
    in2 = []
    for i in range(NCORES):
        tau_in = np.ascontiguousarray(
            np.broadcast_to(tau[i * BL : (i + 1) * BL], (128, BL))
        )
        in2.append(
            {
                "acts": acts_sh[i],
                "Wd": W_dec,
                "bd": b_dec,
                "thr": thr_in,
                "tau": tau_in,
                "deadc": deadc_in,
            }
        )
    r2 = _run_spmd(l2, in2, trace=bool(_timing is not None))
    if _timing is not None:
        _timing.append(("l2", r2.exec_time_ns))

    # ---------------- host: unshard + tie patches + losses ----------------
    acts_topk = np.empty((B, J), dtype=np.float32)
    x_rec = np.empty((B, D), dtype=np.float32)
    x_aux = np.empty((B, D), dtype=np.float32)
    for i in range(NCORES):
        acts_topk[i * BL : (i + 1) * BL] = r2.results[i]["topk_o"].T
        x_rec[i * BL : (i + 1) * BL] = r2.results[i]["xrec"].T
        x_aux[i * BL : (i + 1) * BL] = r2.results[i]["xaux"].T
    for _, row, j in drops:
        acts_topk[row, j] = 0.0
        x_rec[row] -= t * W_dec[j]

    l2_loss = np.float32(np.mean((x_rec - x).astype(np.float64) ** 2))
    l1_norm = np.float32(acts_topk.sum(dtype=np.float64) / B)
    l1_loss = np.float32(0.0)
    l0_norm = np.float32(count_kept / B)
    residual = x - x_rec
    aux_l2 = np.float32(
        AUX_PEN * np.mean((x_aux - residual).astype(np.float64) ** 2)
    )
    n_dead = int((nbna >= N_DEAD).sum())
    aux_loss = aux_l2 if n_dead > 0 else np.float32(0.0)
    loss = np.float32(l2_loss + l1_loss + aux_loss)
    num_dead_features = np.int32((nbna > N_DEAD).sum())

    return (
        x_rec,
        acts_topk,
        loss,
        l2_loss,
        l1_loss,
        l0_norm,
        l1_norm,
        aux_loss,
        num_dead_features,
    )


# revision 11
# speedup vs baseline: 1.1990x; 1.1131x over previous
"""BatchTopK SAE forward pass on 8 Trainium2 NeuronCores.

Strategy (data-parallel over the batch, weights replicated):
  Launch 1 (per core): fp32 encoder  acts[j,b] = relu((x-b_dec) @ W_enc + b_enc)
      on a 512-row batch shard, laid out transposed [d_sae, batch_local] so the
      contraction dim is on partitions for both matmul phases.
  Host (gather point between launches): exact global top-(K*B) threshold from
      the device-produced acts (np.partition), index-ordered tie resolution,
      and exact per-row 32nd-largest dead-masked threshold for the aux loss.
  Launch 2 (per core): masked decode  x_rec = (acts>=t)*acts @ W_dec + b_dec
      and aux decode x_aux = (dead & acts>=tau_row)*acts @ W_dec, fp32r matmuls.
  Host: unshard/transpose outputs, apply rare tie patches, reduce the losses.
"""

import sys

sys.path.insert(0, "/opt/trn_rl_repo")

import numpy as np

B = 4096
D = 2048
J = 16384
K = 64
T = K * B
K_AUX = 32
N_DEAD = 10
AUX_PEN = 1.0 / 32.0
NCORES = 8
BL = B // NCORES  # 512 rows per core
DC = D // 128  # 16 contraction chunks (encoder)
JC = J // 128  # 128 d_sae chunks
MB = 4  # decoder m-chunks held in PSUM at once (x4 = 8 banks with aux)
NBLK = D // (128 * MB)  # 4 decoder blocks

_cache = {}


def _f32():
    import concourse.mybir as mybir

    return mybir.dt.float32


def _build_l1():
    import concourse.mybir as mybir
    from concourse.bacc import Bacc
    from concourse.tile import TileContext

    f32 = mybir.dt.float32
    nc = Bacc()
    xT = nc.dram_tensor("xT", [D, BL], f32, kind="ExternalInput")
    We = nc.dram_tensor("We", [D, J], f32, kind="ExternalInput")
    be = nc.dram_tensor("be", [J], f32, kind="ExternalInput")
    bd = nc.dram_tensor("bd", [D], f32, kind="ExternalInput")
    acts = nc.dram_tensor("acts", [J, BL], f32, kind="ExternalOutput")

    with TileContext(nc) as tc:
        with (
            tc.tile_pool(name="xpool", bufs=1) as xpool,
            tc.tile_pool(name="wpool", bufs=3) as wpool,
            tc.tile_pool(name="opool", bufs=3) as opool,
            tc.tile_pool(name="ppool", bufs=2, space="PSUM") as ppool,
        ):
            xs = xpool.tile([128, DC, BL], f32)
            nc.sync.dma_start(out=xs, in_=xT[:, :].rearrange("(c p) b -> p c b", p=128))
            bds = xpool.tile([128, DC], f32)
            nc.sync.dma_start(out=bds, in_=bd[:].rearrange("(c p) -> p c", p=128))
            bes = xpool.tile([128, JC], f32)
            nc.sync.dma_start(out=bes, in_=be[:].rearrange("(c p) -> p c", p=128))
            for c in range(DC):
                nc.vector.tensor_scalar_sub(xs[:, c, :], xs[:, c, :], bds[:, c : c + 1])
            for jc in range(JC):
                ws = wpool.tile([128, DC, 128], f32, tag="ws")
                nc.sync.dma_start(
                    out=ws,
                    in_=We[:, jc * 128 : (jc + 1) * 128].rearrange(
                        "(c p) j -> p c j", p=128
                    ),
                )
                acc = ppool.tile([128, BL], f32, tag="acc")
                for c in range(DC):
                    nc.tensor.matmul(
                        acc,
                        ws[:, c, :],
                        xs[:, c, :],
                        start=(c == 0),
                        stop=(c == DC - 1),
                    )
                ot = opool.tile([128, BL], f32, tag="ot")
                nc.scalar.activation(
                    ot,
                    acc,
                    mybir.ActivationFunctionType.Relu,
                    bias=bes[:, jc : jc + 1],
                    scale=1.0,
                )
                nc.sync.dma_start(out=acts[jc * 128 : (jc + 1) * 128, :], in_=ot)
    nc.finalize()
    return nc


def _build_l2():
    import concourse.mybir as mybir
    from concourse.bacc import Bacc
    from concourse.tile import TileContext

    f32 = mybir.dt.float32
    f32r = mybir.dt.float32r
    nc = Bacc()
    acts = nc.dram_tensor("acts", [J, BL], f32, kind="ExternalInput")
    Wd = nc.dram_tensor("Wd", [J, D], f32, kind="ExternalInput")
    bd = nc.dram_tensor("bd", [D], f32, kind="ExternalInput")
    thr = nc.dram_tensor("thr", [128, 1], f32, kind="ExternalInput")
    tau = nc.dram_tensor("tau", [128, BL], f32, kind="ExternalInput")
    deadc = nc.dram_tensor("deadc", [128, JC], f32, kind="ExternalInput")
    topk_o = nc.dram_tensor("topk_o", [J, BL], f32, kind="ExternalOutput")
    xrec = nc.dram_tensor("xrec", [D, BL], f32, kind="ExternalOutput")
    xaux = nc.dram_tensor("xaux", [D, BL], f32, kind="ExternalOutput")

    with TileContext(nc) as tc:
        with (
            tc.tile_pool(name="cpool", bufs=1) as cpool,
            tc.tile_pool(name="apool", bufs=5) as apool,
            tc.tile_pool(name="wpool", bufs=5) as wpool,
            tc.tile_pool(name="opool", bufs=3) as opool,
            tc.tile_pool(name="ppool", bufs=1, space="PSUM") as ppool,
        ):
            thr_s = cpool.tile([128, 1], f32)
            nc.sync.dma_start(out=thr_s, in_=thr[:, :])
            tau_s = cpool.tile([128, BL], f32)
            nc.sync.dma_start(out=tau_s, in_=tau[:, :])
            deadc_s = cpool.tile([128, JC], f32)
            nc.sync.dma_start(out=deadc_s, in_=deadc[:, :])
            bds = cpool.tile([128, DC], f32)
            nc.sync.dma_start(out=bds, in_=bd[:].rearrange("(c p) -> p c", p=128))

            for blk in range(NBLK):
                rec_ps = [
                    ppool.tile([128, BL], f32, name=f"rec{mi}", tag=f"rec{mi}")
                    for mi in range(MB)
                ]
                aux_ps = [
                    ppool.tile([128, BL], f32, name=f"aux{mi}", tag=f"aux{mi}")
                    for mi in range(MB)
                ]
                for jc in range(JC):
                    at = apool.tile([128, BL], f32, tag="at")
                    nc.sync.dma_start(
                        out=at, in_=acts[jc * 128 : (jc + 1) * 128, :]
                    )
                    wt = wpool.tile([128, MB * 128], f32r, tag="wt")
                    nc.sync.dma_start(
                        out=wt,
                        in_=Wd[
                            jc * 128 : (jc + 1) * 128,
                            blk * MB * 128 : (blk + 1) * MB * 128,
                        ].bitcast(f32r),
                    )
                    # topk = (acts >= t) * acts
                    tk = apool.tile([128, BL], f32r, tag="tk")
                    nc.vector.scalar_tensor_tensor(
                        tk,
                        at,
                        thr_s[:, 0:1],
                        at,
                        mybir.AluOpType.is_ge,
                        mybir.AluOpType.mult,
                    )
                    if blk == 0:
                        nc.sync.dma_start(
                            out=topk_o[jc * 128 : (jc + 1) * 128, :],
                            in_=tk.bitcast(f32),
                        )
                    # qm = (acts * dead[j]) >= tau[b] ; aux = qm * acts
                    qm = apool.tile([128, BL], f32, tag="qm")
                    nc.vector.scalar_tensor_tensor(
                        qm,
                        at,
                        deadc_s[:, jc : jc + 1],
                        tau_s,
                        mybir.AluOpType.mult,
                        mybir.AluOpType.is_ge,
                    )
                    ax = apool.tile([128, BL], f32r, tag="ax")
                    nc.vector.tensor_mul(ax, qm, at)
                    for mi in range(MB):
                        w_r = wt[:, mi * 128 : (mi + 1) * 128]
                        nc.tensor.matmul(
                            rec_ps[mi], w_r, tk, start=(jc == 0), stop=(jc == JC - 1)
                        )
                        nc.tensor.matmul(
                            aux_ps[mi], w_r, ax, start=(jc == 0), stop=(jc == JC - 1)
                        )
                for mi in range(MB):
                    mc = blk * MB + mi
                    rt = opool.tile([128, BL], f32, tag="rt")
                    nc.scalar.activation(
                        rt,
                        rec_ps[mi],
                        mybir.ActivationFunctionType.Identity,
                        bias=bds[:, mc : mc + 1],
                        scale=1.0,
                    )
                    nc.sync.dma_start(out=xrec[mc * 128 : (mc + 1) * 128, :], in_=rt)
                    au = opool.tile([128, BL], f32, tag="au")
                    nc.scalar.copy(au, aux_ps[mi])
                    nc.sync.dma_start(out=xaux[mc * 128 : (mc + 1) * 128, :], in_=au)
    nc.finalize()
    return nc


def _get_kernels():
    if "l1" not in _cache:
        _cache["l1"] = _build_l1()
        _cache["l2"] = _build_l2()
    return _cache["l1"], _cache["l2"]


def _run_spmd(nc, in_maps, trace=False):
    from concourse.bass_utils import run_bass_kernel_spmd

    if trace:
        try:
            return run_bass_kernel_spmd(
                nc, in_maps, core_ids=list(range(NCORES)), trace=True
            )
        except Exception as e:  # profiling infra unavailable -> run untraced
            print(f"trace run failed ({type(e).__name__}: {e}); rerunning untraced")
    return run_bass_kernel_spmd(nc, in_maps, core_ids=list(range(NCORES)))


def _numpy_fallback(x, W_dec, b_dec, nbna, acts):
    """Exact numpy evaluation downstream of acts (degenerate inputs only)."""
    flat = acts.ravel()
    idx = np.argsort(-flat, kind="stable")[:T]
    topk = np.zeros_like(flat)
    topk[idx] = flat[idx]
    acts_topk = topk.reshape(acts.shape)
    x_rec = (acts_topk @ W_dec + b_dec).astype(np.float32)
    l2_loss = np.float32(np.mean((x_rec - x).astype(np.float64) ** 2))
    l1_norm = np.float32(acts_topk.sum(dtype=np.float64) / B)
    l0_norm = np.float32((acts_topk > 0).sum() / B)
    dead = nbna >= N_DEAD
    residual = x - x_rec
    masked = np.where(dead[None, :], acts, -1.0)
    sel = np.argsort(-masked, axis=1, kind="stable")[:, :K_AUX]
    acts_aux = np.zeros_like(acts)
    rows = np.arange(B)[:, None]
    acts_aux[rows, sel] = np.maximum(masked[rows, sel], 0.0)
    x_aux = (acts_aux @ W_dec).astype(np.float32)
    aux_l2 = np.float32(AUX_PEN * np.mean((x_aux - residual).astype(np.float64) ** 2))
    n_dead = int(dead.sum())
    aux_loss = aux_l2 if n_dead > 0 else np.float32(0.0)
    loss = np.float32(l2_loss + aux_loss)
    return (
        x_rec,
        acts_topk.astype(np.float32),
        loss,
        l2_loss,
        np.float32(0.0),
        l0_norm,
        l1_norm,
        aux_loss,
        np.int32((nbna > N_DEAD).sum()),
    )


def kernel(x, W_enc, W_dec, b_enc, b_dec, num_batches_not_active, _timing=None):
    x = np.ascontiguousarray(x, dtype=np.float32)
    W_enc = np.ascontiguousarray(W_enc, dtype=np.float32)
    W_dec = np.ascontiguousarray(W_dec, dtype=np.float32)
    b_enc = np.ascontiguousarray(b_enc, dtype=np.float32)
    b_dec = np.ascontiguousarray(b_dec, dtype=np.float32)
    nbna = np.asarray(num_batches_not_active)

    l1, l2 = _get_kernels()

    # ---------------- launch 1: encoder ----------------
    xT = [np.ascontiguousarray(x[i * BL : (i + 1) * BL].T) for i in range(NCORES)]
    in1 = [{"xT": xT[i], "We": W_enc, "be": b_enc, "bd": b_dec} for i in range(NCORES)]
    r1 = _run_spmd(l1, in1, trace=bool(_timing is not None))
    acts_sh = [r1.results[i]["acts"] for i in range(NCORES)]  # [J, BL] each
    if _timing is not None:
        _timing.append(("l1", r1.exec_time_ns))

    # ---------------- host: exact global top-T threshold ----------------
    allv = np.concatenate([a.ravel() for a in acts_sh])
    t = np.partition(allv, allv.size - T)[allv.size - T]
    count_gt = int((allv > t).sum())
    needed = T - count_gt  # >= 1 by construction
    del allv
    if not (t > 0.0):
        # Degenerate input (threshold at/below relu floor): fall back to a
        # plain numpy evaluation of the whole forward pass for correctness.
        return _numpy_fallback(
            x, W_dec, b_dec, nbna, np.concatenate([a.T for a in acts_sh], axis=0)
        )
    # tie positions in reference flat order: flat = (global_row)*J + j
    tie_flat = []
    for i in range(NCORES):
        jj, bb = np.nonzero(acts_sh[i] == t)
        rows = i * BL + bb
        tie_flat.extend(zip(rows * J + jj, rows, jj))
    tie_flat.sort()
    drops = tie_flat[needed:]  # (flat, row, j) to zero out, usually empty
    count_kept = count_gt + needed

    # ---------------- host: per-row aux threshold (exact 32nd largest) ----
    dead = (nbna >= N_DEAD).astype(np.float32)  # [J]
    tau = np.empty((B,), dtype=np.float32)
    for i in range(NCORES):
        masked = np.where(dead[:, None] > 0, acts_sh[i], -1.0)  # [J, BL]
        tau[i * BL : (i + 1) * BL] = np.partition(masked, J - K_AUX, axis=0)[
            J - K_AUX, :
        ]
        del masked
    tau = np.maximum(tau, np.float32(1e-30))  # tau<=0 => select all positive dead

    # ---------------- launch 2: masked decode + aux decode ----------------
    thr_in = np.full((128, 1), t, dtype=np.float32)
    deadc_in = np.ascontiguousarray(dead.reshape(JC, 128).T)  # [128, JC]
    in2 = []
    for i in range(NCORES):
        tau_in = np.ascontiguousarray(
            np.broadcast_to(tau[i * BL : (i + 1) * BL], (128, BL))
        )
        in2.append(
            {
                "acts": acts_sh[i],
                "Wd": W_dec,
                "bd": b_dec,
                "thr": thr_in,
                "tau": tau_in,
                "deadc": deadc_in,
            }
        )
    r2 = _run_spmd(l2, in2, trace=bool(_timing is not None))
    if _timing is not None:
        _timing.append(("l2", r2.exec_time_ns))

    # ---------------- host: unshard + tie patches + losses ----------------
    acts_topk = np.empty((B, J), dtype=np.float32)
    x_rec = np.empty((B, D), dtype=np.float32)
    x_aux = np.empty((B, D), dtype=np.float32)
    for i in range(NCORES):
        acts_topk[i * BL : (i + 1) * BL] = r2.results[i]["topk_o"].T
        x_rec[i * BL : (i + 1) * BL] = r2.results[i]["xrec"].T
        x_aux[i * BL : (i + 1) * BL] = r2.results[i]["xaux"].T
    for _, row, j in drops:
        acts_topk[row, j] = 0.0
        x_rec[row] -= t * W_dec[j]

    l2_loss = np.float32(np.mean((x_rec - x).astype(np.float64) ** 2))
    l1_norm = np.float32(acts_topk.sum(dtype=np.float64) / B)
    l1_loss = np.float32(0.0)
    l0_norm = np.float32(count_kept / B)
    residual = x - x_rec
    aux_l2 = np.float32(
        AUX_PEN * np.mean((x_aux - residual).astype(np.float64) ** 2)
    )
    n_dead = int((nbna >= N_DEAD).sum())
    aux_loss = aux_l2 if n_dead > 0 else np.float32(0.0)
    loss = np.float32(l2_loss + l1_loss + aux_loss)
    num_dead_features = np.int32((nbna > N_DEAD).sum())

    return (
        x_rec,
        acts_topk,
        loss,
        l2_loss,
        l1_loss,
        l0_norm,
        l1_norm,
        aux_loss,
        num_dead_features,
    )
